# revision 7
# baseline (speedup 1.0000x reference)
"""BiLSTM-CRF Trainium2 kernel (8 NeuronCores, SPMD).

Strategy
--------
T=8192 timesteps, batch=1.  The LSTM recurrence is sequential, but the
state forgets exponentially (forget gates ~= 0.5), so the sequence is
split into chunks processed in parallel "lanes", each lane warming up
for W steps from zero state before emitting its chunk.  Empirically the
warm state converges to ~1 ulp of the true state after ~64 steps; W=96.

Each core handles a contiguous 1024-step block, both LSTM directions
interleaved, with 36 lanes x 32-step chunks per direction (4 extra lanes
cover the +-64 halo needed by the Viterbi warmup).  Gate-major layout:
hidden/gate units on partitions, lanes on the free dim, so the per-step
matvec h @ W_hh.T becomes 16 accumulating 128x128xL matmuls and the gate
nonlinearities are wide DVE/ACT ops.

The Viterbi forward recurrence is max-plus and also forgets its initial
vector (up to a rank-1 shift that argmax ignores), so it is chunked the
same way: fv (forward) and bv (backward suffix) scans, 64 lanes x
16-step chunks, warmup 64.  best_path[t] = argmax(fv_t + bv_t); the path
score is re-summed on the host in the reference's accumulation order.

The embedding table is trimmed host-side to the unique rows each core's
window references (index bookkeeping only); the gather itself runs on
device via indirect DMA.
"""

import sys

sys.path.insert(0, "/opt/trn_rl_repo")

import numpy as np

import concourse.bass as bass
import concourse.mybir as mybir
from concourse import bacc
from concourse.bass import AP, IndirectOffsetOnAxis
from concourse.bass_utils import run_bass_kernel_spmd
from concourse.masks import make_identity
from concourse.tile import TileContext

F32 = mybir.dt.float32
I32 = mybir.dt.int32
ALU = mybir.AluOpType
ACTF = mybir.ActivationFunctionType

NCORES = 8
T, D, HD, K, V = 8192, 300, 256, 19, 100000
TCORE = T // NCORES  # 1024
START, STOP, NEG = K - 2, K - 1, -10000.0

# LSTM lane geometry
W_L = 96          # warmup steps
C_L = 32          # chunk length per lane
L_L = 36          # lanes per direction per core (32 main + 4 halo)
S_L = W_L + C_L   # 128 steps per lane
TBUF = 1344       # xg buffer: tau -> global t = base - 160 + tau
NB = 11           # gather batches of 128 rows (1408 >= TBUF)
NROWS = NB * 128  # 1408
TF = 1216         # houtT free size; tau_f -> t = base - 64 + tau_f, valid [0,1152)
NFE = 1152        # feats halo range size

# Viterbi lane geometry
W_V = 64
C_V = 16
L_V = 64
S_V = W_V + C_V   # 80
NSF = 80          # fv feats slabs: tau_f = 16*l + s',  s' in [0,80)
NSB = 82          # bv feats slabs: tau_f = 16*l + 64 + s'', s'' in [0,82)

# gate slot order in PSUM: i0 i1 f0 f1 o0 o1 g0 g1 (orig chunk indices)
SLOT_ORIG = [0, 1, 2, 3, 6, 7, 4, 5]

_PROGRAM_CACHE = {}


def _bcast_mid(ap_2d: AP, n: int) -> AP:
    """[P, m] -> [P, n, m] with the new middle dim broadcast (stride 0)."""
    return AP(ap_2d.tensor, ap_2d.offset, [ap_2d.ap[0], [0, n], ap_2d.ap[-1]])


def _build_program():
    nc = bacc.Bacc("TRN2", target_bir_lowering=False, debug=False,
                   num_devices=NCORES)

    def din(name, shape, dt=F32):
        return nc.dram_tensor(name, shape, dt, kind="ExternalInput").ap()

    def dout(name, shape, dt=F32):
        return nc.dram_tensor(name, shape, dt, kind="ExternalOutput").ap()

    emb_sub = din("emb_sub", [NROWS, D])
    ids = din("ids", [NROWS], I32)
    w_ihT = din("w_ihT", [128, 2 * 3 * 1024])      # [p, dir*3072 + kc*1024 + slot*128 + r]
    whhT = din("whhT", [128, 2 * 2 * 1024])        # [p, dir*2048 + kc*1024 + slot*128 + r]
    bias = din("bias", [128, 16])                  # [p, dir*8 + slot]
    w_outT = din("w_outT", [128, 4 * K])           # [p, kc*19 + j]
    trans_rep = din("trans_rep", [128, K * K])     # [p, j*19+i] = trans'[j,i]
    transT_rep = din("transT_rep", [128, K * K])   # [p, i*19+j] = trans'[j,i]
    lmask = din("lmask", [128, 2 * 2 * L_L])       # [p, dir*72 + kc*36 + l]
    lhval = din("lhval", [128, 2 * 2 * L_L])
    lcval = din("lcval", [128, 2 * 2 * L_L])
    vmask = din("vmask", [128, 2 * K])             # [l, scan*19 + j]  (scan 0=fv,1=bv)
    vval = din("vval", [128, 2 * K])

    feats_o = dout("feats_o", [TCORE, K])
    fv_o = dout("fv_o", [TCORE, K])
    bv_o = dout("bv_o", [TCORE, K])

    from contextlib import ExitStack
    with TileContext(nc) as tc, ExitStack() as stack:
        cp = stack.enter_context(tc.tile_pool(name="const", bufs=1))
        bp = stack.enter_context(tc.tile_pool(name="big", bufs=1))

        ident = cp.tile([128, 128], F32)
        make_identity(nc, ident[:])

        w_outT_sb = cp.tile([128, 4 * K], F32)
        nc.sync.dma_start(out=w_outT_sb[:], in_=w_outT)
        trans_sb = cp.tile([128, K * K], F32)
        nc.sync.dma_start(out=trans_sb[:], in_=trans_rep)
        transT_sb = cp.tile([128, K * K], F32)
        nc.sync.dma_start(out=transT_sb[:], in_=transT_rep)
        bias_sb = cp.tile([128, 16], F32)
        nc.sync.dma_start(out=bias_sb[:], in_=bias)
        whhT_sb = cp.tile([128, 2 * 2048], F32)
        nc.sync.dma_start(out=whhT_sb[:], in_=whhT)
        lmask_sb = cp.tile([128, 4 * L_L], F32)
        nc.sync.dma_start(out=lmask_sb[:], in_=lmask)
        lhval_sb = cp.tile([128, 4 * L_L], F32)
        nc.sync.dma_start(out=lhval_sb[:], in_=lhval)
        lcval_sb = cp.tile([128, 4 * L_L], F32)
        nc.sync.dma_start(out=lcval_sb[:], in_=lcval)
        vmask_sb = cp.tile([128, 2 * K], F32)
        nc.sync.dma_start(out=vmask_sb[:], in_=vmask)
        vval_sb = cp.tile([128, 2 * K], F32)
        nc.sync.dma_start(out=vval_sb[:], in_=vval)

        # persistent big buffers
        xgT = [bp.tile([128, 8 * TBUF], F32, tag=f"xgT{d}", name=f"xgT{d}") for d in range(2)]
        houtT = [bp.tile([128, 2 * TF], F32, tag=f"houtT{d}", name=f"houtT{d}") for d in range(2)]
        featsV_f = bp.tile([128, NSF * K], F32)
        featsV_b = bp.tile([128, NSB * K], F32)
        h_scr = [bp.tile([128, 2 * L_L], F32, tag=f"hscr{d}", name=f"hscr{d}") for d in range(2)]
        c_sb = [bp.tile([128, 2 * L_L], F32, tag=f"csb{d}", name=f"csb{d}") for d in range(2)]
        g_sb = [bp.tile([128, 8 * L_L], F32, tag=f"gsb{d}", name=f"gsb{d}") for d in range(2)]
        a_sb = [bp.tile([128, 8 * L_L], F32, tag=f"asb{d}", name=f"asb{d}") for d in range(2)]
        tg_sb = [bp.tile([128, 2 * L_L], F32, tag=f"tgsb{d}", name=f"tgsb{d}") for d in range(2)]
        tc_sb = [bp.tile([128, 2 * L_L], F32, tag=f"tcsb{d}", name=f"tcsb{d}") for d in range(2)]

        for d in range(2):
            nc.gpsimd.memset(houtT[d][:], 0.0)
            nc.gpsimd.memset(h_scr[d][:], 0.0)
            nc.gpsimd.memset(c_sb[d][:], 0.0)

        # ---- phase 1: gather + transpose + input projection ----
        with tc.tile_pool(name="ph1", bufs=1) as p1, \
             tc.tile_pool(name="ps_tr", bufs=4, space="PSUM") as pstr, \
             tc.tile_pool(name="ps_proj", bufs=2, space="PSUM") as psproj:
            ids_sb = p1.tile([128, NB], I32)
            nc.sync.dma_start(out=ids_sb[:],
                              in_=ids.rearrange("(b p) -> p b", p=128))
            x_sb = p1.tile([128, NB * D], F32)
            for b in range(NB):
                nc.gpsimd.indirect_dma_start(
                    out=x_sb[:, b * D:(b + 1) * D],
                    out_offset=None,
                    in_=emb_sub,
                    in_offset=IndirectOffsetOnAxis(ap=ids_sb[:, b:b + 1], axis=0),
                )
            xT_sb = p1.tile([128, 3 * NROWS], F32)
            for b in range(NB):
                for kc in range(3):
                    w = min(128, D - kc * 128)
                    pt = pstr.tile([128, 128], F32, name="pt", tag="pt")
                    nc.tensor.transpose(
                        out=pt[:w, :],
                        in_=x_sb[:, b * D + kc * 128: b * D + kc * 128 + w],
                        identity=ident[:],
                    )
                    nc.vector.tensor_copy(
                        out=xT_sb[:w, kc * NROWS + b * 128: kc * NROWS + (b + 1) * 128],
                        in_=pt[:w, :])

            w_ihT_sb = p1.tile([128, 2 * 3072], F32)
            nc.sync.dma_start(out=w_ihT_sb[:], in_=w_ihT)

            ntiles = [(0, 512), (512, 512), (1024, 320)]
            for d in range(2):
                for slot in range(8):
                    for (n0, nn) in ntiles:
                        pp = psproj.tile([128, 512], F32, tag="proj", name="pp")
                        for kc in range(3):
                            nc.tensor.matmul(
                                out=pp[:, :nn],
                                lhsT=w_ihT_sb[:, d * 3072 + kc * 1024 + slot * 128:
                                              d * 3072 + kc * 1024 + (slot + 1) * 128],
                                rhs=xT_sb[:, kc * NROWS + n0: kc * NROWS + n0 + nn],
                                start=(kc == 0), stop=(kc == 2),
                            )
                        nc.scalar.activation(
                            out=xgT[d][:, slot * TBUF + n0: slot * TBUF + n0 + nn],
                            in_=pp[:, :nn],
                            func=ACTF.Identity,
                            bias=bias_sb[:, d * 8 + slot: d * 8 + slot + 1],
                        )

        # ---- phase 2: LSTM recurrence, both dirs interleaved ----
        with tc.tile_pool(name="ps_rec", bufs=4, space="PSUM") as psrec:
            xgT_v = [xgT[d][:].rearrange("p (m t) -> p m t", m=8) for d in range(2)]
            houtT_v = [houtT[d][:].rearrange("p (k t) -> p k t", k=2) for d in range(2)]

            def h_prev_ap(d, s, kc):
                if s == 0 or s - 1 < W_L:
                    return h_scr[d][:, kc * L_L:(kc + 1) * L_L]
                sp = s - 1
                off = (sp - W_L) if d == 0 else (127 - sp)
                return houtT_v[d][:, kc, off: off + 32 * (L_L - 1) + 1: 32]

            for s in range(S_L):
                for d in range(2):
                    if s == W_L:
                        # inject true initial state into the designated lane
                        for tgt, val in ((h_scr[d], lhval_sb), (c_sb[d], lcval_sb)):
                            nc.vector.tensor_tensor(
                                out=tgt[:], in0=tgt[:],
                                in1=lmask_sb[:, d * 72:(d + 1) * 72], op=ALU.mult)
                            nc.vector.tensor_tensor(
                                out=tgt[:], in0=tgt[:],
                                in1=val[:, d * 72:(d + 1) * 72], op=ALU.add)
                    ps = psrec.tile([128, 8 * L_L], F32, tag=f"ps{d}", name=f"ps{d}")
                    for m in range(8):
                        for kc in range(2):
                            nc.tensor.matmul(
                                out=ps[:, m * L_L:(m + 1) * L_L],
                                lhsT=whhT_sb[:, d * 2048 + kc * 1024 + m * 128:
                                             d * 2048 + kc * 1024 + (m + 1) * 128],
                                rhs=h_prev_ap(d, s, kc),
                                start=(kc == 0), stop=(kc == 1),
                            )
                    # g = ghh + xg[t(s)]  (xg cols strided by lane)
                    off = s if d == 0 else (223 - s)
                    xg_ap = xgT_v[d][:, :, off: off + 32 * (L_L - 1) + 1: 32]
                    g3 = g_sb[d][:].rearrange("p (m l) -> p m l", m=8)
                    nc.vector.tensor_tensor(out=g3, in0=ps[:].rearrange(
                        "p (m l) -> p m l", m=8), in1=xg_ap, op=ALU.add)
                    # activations: slots [i0 i1 f0 f1 o0 o1] sigmoid, [g0 g1] tanh
                    nc.scalar.activation(out=a_sb[d][:, :6 * L_L],
                                         in_=g_sb[d][:, :6 * L_L], func=ACTF.Sigmoid)
                    nc.scalar.activation(out=a_sb[d][:, 6 * L_L:],
                                         in_=g_sb[d][:, 6 * L_L:], func=ACTF.Tanh)
                    i_ap = a_sb[d][:, 0:2 * L_L]
                    f_ap = a_sb[d][:, 2 * L_L:4 * L_L]
                    o_ap = a_sb[d][:, 4 * L_L:6 * L_L]
                    gg_ap = a_sb[d][:, 6 * L_L:8 * L_L]
                    nc.vector.tensor_tensor(out=tg_sb[d][:], in0=i_ap, in1=gg_ap,
                                            op=ALU.mult)
                    nc.vector.tensor_tensor(out=c_sb[d][:], in0=f_ap, in1=c_sb[d][:],
                                            op=ALU.mult)
                    nc.vector.tensor_tensor(out=c_sb[d][:], in0=c_sb[d][:],
                                            in1=tg_sb[d][:], op=ALU.add)
                    nc.scalar.activation(out=tc_sb[d][:], in_=c_sb[d][:],
                                         func=ACTF.Tanh)
                    # h -> scratch (warmup) or houtT strided slice
                    if s < W_L:
                        h_out = h_scr[d][:].rearrange("p (k l) -> p k l", k=2)
                    else:
                        off_o = (s - W_L) if d == 0 else (127 - s)
                        h_out = houtT_v[d][:, :, off_o: off_o + 32 * (L_L - 1) + 1: 32]
                    nc.vector.tensor_tensor(out=h_out, in0=o_ap.rearrange(
                        "p (k l) -> p k l", k=2), in1=tc_sb[d][:].rearrange(
                        "p (k l) -> p k l", k=2), op=ALU.mult)

        # ---- phase 3: feats slabs for viterbi ----
        with tc.tile_pool(name="ps_fe", bufs=4, space="PSUM") as psfe:
            for which, nsl, base_off, dst in ((0, NSF, 0, featsV_f),
                                              (1, NSB, 64, featsV_b)):
                for sl in range(nsl):
                    t0 = base_off + sl
                    pf = psfe.tile([64, K], F32, tag="fe", name="pf")
                    for kc in range(4):
                        d, sub = kc // 2, kc % 2
                        lhs = houtT_v[d][:, sub, t0: t0 + 16 * 63 + 1: 16]
                        nc.tensor.matmul(
                            out=pf[:, :], lhsT=lhs,
                            rhs=w_outT_sb[:, kc * K:(kc + 1) * K],
                            start=(kc == 0), stop=(kc == 3),
                        )
                    nc.scalar.copy(out=dst[0:64, sl * K:(sl + 1) * K], in_=pf[:, :])

        # ---- phase 4: viterbi fv/bv scans ----
        with tc.tile_pool(name="ph4", bufs=1) as p4:
            m_sb = p4.tile([64, K * K], F32)
            mb_sb = p4.tile([64, K * K], F32)
            red_sb = p4.tile([64, K], F32)
            u_sb = p4.tile([64, K], F32)
            fvA = p4.tile([64, K], F32)
            bvA = p4.tile([64, K], F32)
            fvs_sb = p4.tile([64, C_V * K], F32)
            bvs_sb = p4.tile([64, C_V * K], F32)
            nc.gpsimd.memset(fvA[:], 0.0)
            nc.gpsimd.memset(bvA[:], 0.0)

            m3 = m_sb[:].rearrange("l (a b) -> l a b", a=K)
            mb3 = mb_sb[:].rearrange("l (a b) -> l a b", a=K)
            tr3 = trans_sb[0:64, :].rearrange("l (a b) -> l a b", a=K)
            trT3 = transT_sb[0:64, :].rearrange("l (a b) -> l a b", a=K)

            fv_prev = fvA[:, :]
            for s in range(S_V):
                if s == W_V:
                    nc.vector.tensor_tensor(out=fv_prev, in0=fv_prev,
                                            in1=vmask_sb[0:64, 0:K], op=ALU.mult)
                    nc.vector.tensor_tensor(out=fv_prev, in0=fv_prev,
                                            in1=vval_sb[0:64, 0:K], op=ALU.add)
                nc.vector.tensor_tensor(out=m3, in0=_bcast_mid(fv_prev, K),
                                        in1=tr3, op=ALU.add)
                nc.vector.tensor_reduce(out=red_sb[:, :], in_=m3,
                                        axis=mybir.AxisListType.X, op=ALU.max)
                nxt = fvs_sb[:, (s - W_V) * K:(s - W_V + 1) * K] if s >= W_V \
                    else fvA[:, :]
                nc.vector.tensor_tensor(out=nxt, in0=red_sb[:, :],
                                        in1=featsV_f[0:64, s * K:(s + 1) * K],
                                        op=ALU.add)
                fv_prev = nxt

            bv_prev = bvA[:, :]
            for s in range(S_V):
                ftb = featsV_b[0:64, (80 - s) * K:(81 - s) * K]
                nc.vector.tensor_tensor(out=u_sb[:, :], in0=bv_prev, in1=ftb,
                                        op=ALU.add)
                nc.vector.tensor_tensor(out=mb3, in0=_bcast_mid(u_sb[:, :], K),
                                        in1=trT3, op=ALU.add)
                nxt = bvs_sb[:, (79 - s) * K:(80 - s) * K] if s >= W_V \
                    else bvA[:, :]
                nc.vector.tensor_reduce(out=nxt, in_=mb3,
                                        axis=mybir.AxisListType.X, op=ALU.max)
                if s == W_V:
                    nc.vector.tensor_tensor(out=nxt, in0=nxt,
                                            in1=vmask_sb[0:64, K:2 * K], op=ALU.mult)
                    nc.vector.tensor_tensor(out=nxt, in0=nxt,
                                            in1=vval_sb[0:64, K:2 * K], op=ALU.add)
                bv_prev = nxt

            # ---- phase 5: outputs ----
            nc.sync.dma_start(
                out=feats_o.rearrange("(l s) j -> l s j", l=64),
                in_=featsV_b[0:64, 0:C_V * K].rearrange("l (s j) -> l s j", s=C_V))
            nc.sync.dma_start(
                out=fv_o.rearrange("(l s) j -> l s j", l=64),
                in_=fvs_sb[:, :].rearrange("l (s j) -> l s j", s=C_V))
            nc.sync.dma_start(
                out=bv_o.rearrange("(l s) j -> l s j", l=64),
                in_=bvs_sb[:, :].rearrange("l (s j) -> l s j", s=C_V))

    nc.compile()
    return nc


def _prep_core(c, sentence, emb, w_ih_f, w_hh_f, b_f, w_ih_b, w_hh_b, b_b,
               w_out, b_out, transitions, h0, c0):
    base = c * TCORE
    tglob = np.arange(NROWS) + base - 160
    tok = sentence[np.clip(tglob, 0, T - 1)]
    uniq, inv = np.unique(tok, return_inverse=True)
    emb_sub = np.zeros((NROWS, D), np.float32)
    emb_sub[:len(uniq)] = emb[uniq]
    ids = inv.astype(np.int32)

    def packT(wm, pad_k):
        # wm [rows, cols] -> [128, nk*1024] with [p, kc*1024 + slot*128 + r]
        nk = (wm.shape[1] + 127) // 128
        outa = np.zeros((128, nk * 1024), np.float32)
        for kc in range(nk):
            w = min(128, wm.shape[1] - kc * 128)
            for slot in range(8):
                rows = wm[SLOT_ORIG[slot] * 128:(SLOT_ORIG[slot] + 1) * 128,
                          kc * 128: kc * 128 + w]
                outa[:w, kc * 1024 + slot * 128: kc * 1024 + (slot + 1) * 128] = rows.T
        return outa

    w_ihT = np.concatenate([packT(w_ih_f, 3), packT(w_ih_b, 3)], axis=1)
    whhT = np.concatenate([packT(w_hh_f, 2), packT(w_hh_b, 2)], axis=1)
    bias = np.zeros((128, 16), np.float32)
    for d, b in ((0, b_f), (1, b_b)):
        for slot in range(8):
            bias[:, d * 8 + slot] = b[SLOT_ORIG[slot] * 128:(SLOT_ORIG[slot] + 1) * 128]
    w_outT = np.zeros((128, 4 * K), np.float32)
    for kc in range(4):
        w_outT[:, kc * K:(kc + 1) * K] = w_out[:, kc * 128:(kc + 1) * 128].T
    transp = (transitions + b_out[:, None]).astype(np.float32)
    trans_rep = np.broadcast_to(transp.reshape(-1), (128, K * K)).copy()
    transT_rep = np.broadcast_to(transp.T.reshape(-1), (128, K * K)).copy()

    lmask = np.ones((128, 4 * L_L), np.float32)
    lhval = np.zeros((128, 4 * L_L), np.float32)
    lcval = np.zeros((128, 4 * L_L), np.float32)
    if c == 0:
        for kc in range(2):
            lmask[:, 0 * 72 + kc * L_L + 2] = 0.0
            lhval[:, 0 * 72 + kc * L_L + 2] = h0[0][kc * 128:(kc + 1) * 128]
            lcval[:, 0 * 72 + kc * L_L + 2] = c0[0][kc * 128:(kc + 1) * 128]
    if c == NCORES - 1:
        for kc in range(2):
            lmask[:, 1 * 72 + kc * L_L + 33] = 0.0
            lhval[:, 1 * 72 + kc * L_L + 33] = h0[1][kc * 128:(kc + 1) * 128]
            lcval[:, 1 * 72 + kc * L_L + 33] = c0[1][kc * 128:(kc + 1) * 128]

    vmask = np.ones((128, 2 * K), np.float32)
    vval = np.zeros((128, 2 * K), np.float32)
    if c == 0:
        vmask[0, 0:K] = 0.0
        vval[0, 0:K] = NEG
        vval[0, START] = 0.0
    if c == NCORES - 1:
        vmask[63, K:2 * K] = 0.0
        vval[63, K:2 * K] = transitions[STOP]

    return dict(emb_sub=emb_sub, ids=ids, w_ihT=w_ihT, whhT=whhT, bias=bias,
                w_outT=w_outT, trans_rep=trans_rep, transT_rep=transT_rep,
                lmask=lmask, lhval=lhval, lcval=lcval, vmask=vmask, vval=vval)


def kernel(sentence, emb, w_ih_f, w_hh_f, b_f, w_ih_b, w_hh_b, b_b,
           w_out, b_out, transitions, h0, c0):
    sentence = np.asarray(sentence)
    emb = np.asarray(emb, np.float32)
    args = (sentence, emb, np.asarray(w_ih_f), np.asarray(w_hh_f),
            np.asarray(b_f), np.asarray(w_ih_b), np.asarray(w_hh_b),
            np.asarray(b_b), np.asarray(w_out), np.asarray(b_out),
            np.asarray(transitions), np.asarray(h0), np.asarray(c0))

    if "nc" not in _PROGRAM_CACHE:
        _PROGRAM_CACHE["nc"] = _build_program()
    nc = _PROGRAM_CACHE["nc"]

    in_maps = [_prep_core(c, *args) for c in range(NCORES)]
    res = run_bass_kernel_spmd(nc, in_maps, core_ids=list(range(NCORES)))

    feats = np.concatenate([res.results[c]["feats_o"] for c in range(NCORES)], 0)
    fv = np.concatenate([res.results[c]["fv_o"] for c in range(NCORES)], 0)
    bv = np.concatenate([res.results[c]["bv_o"] for c in range(NCORES)], 0)

    b_out_np = np.asarray(b_out, np.float32)
    trans_np = np.asarray(transitions, np.float32)
    feats = feats + b_out_np[None, :]

    tot = fv + bv
    path = tot.argmax(axis=1).astype(np.int32)

    # path score accumulated in the reference's order/precision
    terms = np.empty(2 * T, np.float32)
    prev = np.concatenate([[START], path[:-1]])
    terms[0::2] = trans_np[path, prev]
    terms[1::2] = feats[np.arange(T), path]
    acc = np.cumsum(terms, dtype=np.float32)[-1]
    score = np.float32(acc + trans_np[STOP, path[-1]])
    return score, path


# revision 12
# speedup vs baseline: 1.6396x; 1.6396x over previous
"""BiLSTM-CRF Trainium2 kernel (8 NeuronCores, SPMD).

Strategy
--------
T=8192 timesteps, batch=1.  The LSTM recurrence is sequential, but the
state forgets exponentially (forget gates ~= 0.5), so the sequence is
split into chunks processed in parallel "lanes", each lane warming up
for W steps from zero state before emitting its chunk.  Empirically the
warm state converges to ~1 ulp of the true state after ~64 steps; W=96.

Each core handles a contiguous 1024-step block, both LSTM directions
interleaved, with 36 lanes x 32-step chunks per direction (4 extra lanes
cover the +-64 halo needed by the Viterbi warmup).  Gate-major layout:
hidden/gate units on partitions, lanes on the free dim, so the per-step
matvec h @ W_hh.T becomes 16 accumulating 128x128xL matmuls and the gate
nonlinearities are wide DVE/ACT ops.

The Viterbi forward recurrence is max-plus and also forgets its initial
vector (up to a rank-1 shift that argmax ignores), so it is chunked the
same way: fv (forward) and bv (backward suffix) scans, 64 lanes x
16-step chunks, warmup 64.  best_path[t] = argmax(fv_t + bv_t); the path
score is re-summed on the host in the reference's accumulation order.

The embedding table is trimmed host-side to the unique rows each core's
window references (index bookkeeping only); the gather itself runs on
device via indirect DMA.
"""

import sys

sys.path.insert(0, "/opt/trn_rl_repo")

import hashlib
import pathlib
import shutil

import numpy as np

import concourse.bass as bass
import concourse.mybir as mybir
from concourse import bacc
from concourse.bass import AP, IndirectOffsetOnAxis
from concourse.bass_utils import run_bass_kernel_spmd
from concourse.masks import make_identity
from concourse.tile import TileContext

# Persistent NEFF cache: the BIR for this kernel is deterministic, so cache
# the walrus-compiled NEFF on disk keyed by BIR hash to skip the multi-minute
# neuronxcc compile in fresh processes.
_NEFF_CACHE_DIR = pathlib.Path("/root/.cache/bass_neff")


def _install_neff_cache():
    import concourse.bass2jax as bass2jax
    if getattr(bass2jax.compile_bir_kernel, "_ant_cached", False):
        return
    orig = bass2jax.compile_bir_kernel

    def cached(bir_json, tmpdir, neff_name="file.neff"):
        try:
            _NEFF_CACHE_DIR.mkdir(parents=True, exist_ok=True)
            h = hashlib.sha256(bir_json).hexdigest()
            cp = _NEFF_CACHE_DIR / f"{h}.neff"
            if cp.exists():
                dst = pathlib.Path(tmpdir) / neff_name
                shutil.copyfile(cp, dst)
                return str(dst)
            p = orig(bir_json, tmpdir, neff_name=neff_name)
            tmp = cp.with_suffix(".tmp")
            shutil.copyfile(p, tmp)
            tmp.replace(cp)
            return p
        except Exception:
            return orig(bir_json, tmpdir, neff_name=neff_name)

    cached._ant_cached = True
    bass2jax.compile_bir_kernel = cached


_install_neff_cache()

F32 = mybir.dt.float32
I32 = mybir.dt.int32
ALU = mybir.AluOpType
ACTF = mybir.ActivationFunctionType

NCORES = 8
T, D, HD, K, V = 8192, 300, 256, 19, 100000
TCORE = T // NCORES  # 1024
START, STOP, NEG = K - 2, K - 1, -10000.0

# LSTM lane geometry
W_L = 96          # warmup steps
C_L = 32          # chunk length per lane
L_L = 36          # lanes per direction per core (32 main + 4 halo)
S_L = W_L + C_L   # 128 steps per lane
TBUF = 1344       # xg buffer: tau -> global t = base - 160 + tau
NB = 11           # gather batches of 128 rows (1408 >= TBUF)
NROWS = NB * 128  # 1408
TF = 1216         # houtT free size; tau_f -> t = base - 64 + tau_f, valid [0,1152)
NFE = 1152        # feats halo range size

# Viterbi lane geometry
W_V = 64
C_V = 16
L_V = 64
S_V = W_V + C_V   # 80
NSF = 80          # fv feats slabs: tau_f = 16*l + s',  s' in [0,80)
NSB = 82          # bv feats slabs: tau_f = 16*l + 64 + s'', s'' in [0,82)

# gate slot order in PSUM: i0 i1 f0 f1 o0 o1 g0 g1 (orig chunk indices)
SLOT_ORIG = [0, 1, 2, 3, 6, 7, 4, 5]

_PROGRAM_CACHE = {}


def _bcast_mid(ap_2d: AP, n: int) -> AP:
    """[P, m] -> [P, n, m] with the new middle dim broadcast (stride 0)."""
    return AP(ap_2d.tensor, ap_2d.offset, [ap_2d.ap[0], [0, n], ap_2d.ap[-1]])


def _build_program():
    nc = bacc.Bacc("TRN2", target_bir_lowering=False, debug=False,
                   num_devices=NCORES)

    def din(name, shape, dt=F32):
        return nc.dram_tensor(name, shape, dt, kind="ExternalInput").ap()

    def dout(name, shape, dt=F32):
        return nc.dram_tensor(name, shape, dt, kind="ExternalOutput").ap()

    emb_sub = din("emb_sub", [NROWS, D])
    ids = din("ids", [NROWS], I32)
    w_ihT = din("w_ihT", [128, 2 * 3 * 1024])      # [p, dir*3072 + kc*1024 + slot*128 + r]
    whhT = din("whhT", [128, 2 * 2 * 1024])        # [p, dir*2048 + kc*1024 + slot*128 + r]
    bias = din("bias", [128, 16])                  # [p, dir*8 + slot]
    w_outT = din("w_outT", [128, 4 * K])           # [p, kc*19 + j]
    trans_rep = din("trans_rep", [128, K * K])     # [p, j*19+i] = trans'[j,i]
    transT_rep = din("transT_rep", [128, K * K])   # [p, i*19+j] = trans'[j,i]
    lmask = din("lmask", [128, 2 * 2 * L_L])       # [p, dir*72 + kc*36 + l]
    lhval = din("lhval", [128, 2 * 2 * L_L])
    lcval = din("lcval", [128, 2 * 2 * L_L])
    vmask = din("vmask", [128, 2 * K])             # [l, scan*19 + j]  (scan 0=fv,1=bv)
    vval = din("vval", [128, 2 * K])

    feats_o = dout("feats_o", [TCORE, K])
    fv_o = dout("fv_o", [TCORE, K])
    bv_o = dout("bv_o", [TCORE, K])

    from contextlib import ExitStack
    with TileContext(nc) as tc, ExitStack() as stack:
        cp = stack.enter_context(tc.tile_pool(name="const", bufs=1))
        bp = stack.enter_context(tc.tile_pool(name="big", bufs=1))

        ident = cp.tile([128, 128], F32)
        make_identity(nc, ident[:])

        w_outT_sb = cp.tile([128, 4 * K], F32)
        nc.sync.dma_start(out=w_outT_sb[:], in_=w_outT)
        trans_sb = cp.tile([128, K * K], F32)
        nc.sync.dma_start(out=trans_sb[:], in_=trans_rep)
        transT_sb = cp.tile([128, K * K], F32)
        nc.sync.dma_start(out=transT_sb[:], in_=transT_rep)
        bias_sb = cp.tile([128, 16], F32)
        nc.sync.dma_start(out=bias_sb[:], in_=bias)
        whhT_sb = cp.tile([128, 2 * 2048], F32)
        nc.sync.dma_start(out=whhT_sb[:], in_=whhT)
        lmask_sb = cp.tile([128, 4 * L_L], F32)
        nc.sync.dma_start(out=lmask_sb[:], in_=lmask)
        lhval_sb = cp.tile([128, 4 * L_L], F32)
        nc.sync.dma_start(out=lhval_sb[:], in_=lhval)
        lcval_sb = cp.tile([128, 4 * L_L], F32)
        nc.sync.dma_start(out=lcval_sb[:], in_=lcval)
        vmask_sb = cp.tile([128, 2 * K], F32)
        nc.sync.dma_start(out=vmask_sb[:], in_=vmask)
        vval_sb = cp.tile([128, 2 * K], F32)
        nc.sync.dma_start(out=vval_sb[:], in_=vval)

        # persistent big buffers
        xgT = [bp.tile([128, 8 * TBUF], F32, tag=f"xgT{d}", name=f"xgT{d}") for d in range(2)]
        houtT = [bp.tile([128, 2 * TF], F32, tag=f"houtT{d}", name=f"houtT{d}") for d in range(2)]
        featsV_f = bp.tile([128, NSF * K], F32)
        featsV_b = bp.tile([128, NSB * K], F32)
        h_scr = [bp.tile([128, 2 * L_L], F32, tag=f"hscr{d}", name=f"hscr{d}") for d in range(2)]
        c_sb = [bp.tile([128, 2 * L_L], F32, tag=f"csb{d}", name=f"csb{d}") for d in range(2)]
        g_sb = [bp.tile([128, 8 * L_L], F32, tag=f"gsb{d}", name=f"gsb{d}") for d in range(2)]
        a_sb = [bp.tile([128, 8 * L_L], F32, tag=f"asb{d}", name=f"asb{d}") for d in range(2)]
        tg_sb = [bp.tile([128, 2 * L_L], F32, tag=f"tgsb{d}", name=f"tgsb{d}") for d in range(2)]
        tc_sb = [bp.tile([128, 2 * L_L], F32, tag=f"tcsb{d}", name=f"tcsb{d}") for d in range(2)]

        for d in range(2):
            nc.gpsimd.memset(houtT[d][:], 0.0)
            nc.gpsimd.memset(h_scr[d][:], 0.0)
            nc.gpsimd.memset(c_sb[d][:], 0.0)

        # ---- phase 1: gather + transpose + input projection ----
        with tc.tile_pool(name="ph1", bufs=1) as p1, \
             tc.tile_pool(name="ps_tr", bufs=4, space="PSUM") as pstr, \
             tc.tile_pool(name="ps_proj", bufs=2, space="PSUM") as psproj:
            ids_sb = p1.tile([128, NB], I32)
            nc.sync.dma_start(out=ids_sb[:],
                              in_=ids.rearrange("(b p) -> p b", p=128))
            x_sb = p1.tile([128, NB * D], F32)
            for b in range(NB):
                nc.gpsimd.indirect_dma_start(
                    out=x_sb[:, b * D:(b + 1) * D],
                    out_offset=None,
                    in_=emb_sub,
                    in_offset=IndirectOffsetOnAxis(ap=ids_sb[:, b:b + 1], axis=0),
                )
            xT_sb = p1.tile([128, 3 * NROWS], F32)
            for b in range(NB):
                for kc in range(3):
                    w = min(128, D - kc * 128)
                    pt = pstr.tile([128, 128], F32, name="pt", tag="pt")
                    nc.tensor.transpose(
                        out=pt[:w, :],
                        in_=x_sb[:, b * D + kc * 128: b * D + kc * 128 + w],
                        identity=ident[:],
                    )
                    nc.vector.tensor_copy(
                        out=xT_sb[:w, kc * NROWS + b * 128: kc * NROWS + (b + 1) * 128],
                        in_=pt[:w, :])

            w_ihT_sb = p1.tile([128, 2 * 3072], F32)
            nc.sync.dma_start(out=w_ihT_sb[:], in_=w_ihT)

            ntiles = [(0, 512), (512, 512), (1024, 320)]
            for d in range(2):
                for slot in range(8):
                    for (n0, nn) in ntiles:
                        pp = psproj.tile([128, 512], F32, tag="proj", name="pp")
                        for kc in range(3):
                            nc.tensor.matmul(
                                out=pp[:, :nn],
                                lhsT=w_ihT_sb[:, d * 3072 + kc * 1024 + slot * 128:
                                              d * 3072 + kc * 1024 + (slot + 1) * 128],
                                rhs=xT_sb[:, kc * NROWS + n0: kc * NROWS + n0 + nn],
                                start=(kc == 0), stop=(kc == 2),
                            )
                        nc.scalar.activation(
                            out=xgT[d][:, slot * TBUF + n0: slot * TBUF + n0 + nn],
                            in_=pp[:, :nn],
                            func=ACTF.Identity,
                            bias=bias_sb[:, d * 8 + slot: d * 8 + slot + 1],
                        )

        # ---- phase 2: LSTM recurrence, both dirs interleaved ----
        with tc.tile_pool(name="ps_rec", bufs=4, space="PSUM") as psrec:
            xgT_v = [xgT[d][:].rearrange("p (m t) -> p m t", m=8) for d in range(2)]
            houtT_v = [houtT[d][:].rearrange("p (k t) -> p k t", k=2) for d in range(2)]

            def h_prev_ap(d, s, kc):
                if s == 0 or s - 1 < W_L:
                    return h_scr[d][:, kc * L_L:(kc + 1) * L_L]
                sp = s - 1
                off = (sp - W_L) if d == 0 else (127 - sp)
                return houtT_v[d][:, kc, off: off + 32 * (L_L - 1) + 1: 32]

            for s in range(S_L):
                for d in range(2):
                    if s == W_L:
                        # inject true initial state into the designated lane
                        for tgt, val in ((h_scr[d], lhval_sb), (c_sb[d], lcval_sb)):
                            nc.vector.tensor_tensor(
                                out=tgt[:], in0=tgt[:],
                                in1=lmask_sb[:, d * 72:(d + 1) * 72], op=ALU.mult)
                            nc.vector.tensor_tensor(
                                out=tgt[:], in0=tgt[:],
                                in1=val[:, d * 72:(d + 1) * 72], op=ALU.add)
                    ps = psrec.tile([128, 8 * L_L], F32, tag=f"ps{d}", name=f"ps{d}")
                    for m in range(8):
                        for kc in range(2):
                            nc.tensor.matmul(
                                out=ps[:, m * L_L:(m + 1) * L_L],
                                lhsT=whhT_sb[:, d * 2048 + kc * 1024 + m * 128:
                                             d * 2048 + kc * 1024 + (m + 1) * 128],
                                rhs=h_prev_ap(d, s, kc),
                                start=(kc == 0), stop=(kc == 1),
                            )
                    # g = ghh + xg[t(s)]  (xg cols strided by lane)
                    off = s if d == 0 else (223 - s)
                    xg_ap = xgT_v[d][:, :, off: off + 32 * (L_L - 1) + 1: 32]
                    g3 = g_sb[d][:].rearrange("p (m l) -> p m l", m=8)
                    nc.vector.tensor_tensor(out=g3, in0=ps[:].rearrange(
                        "p (m l) -> p m l", m=8), in1=xg_ap, op=ALU.add)
                    # activations: slots [i0 i1 f0 f1 o0 o1] sigmoid, [g0 g1] tanh
                    nc.scalar.activation(out=a_sb[d][:, :6 * L_L],
                                         in_=g_sb[d][:, :6 * L_L], func=ACTF.Sigmoid)
                    nc.scalar.activation(out=a_sb[d][:, 6 * L_L:],
                                         in_=g_sb[d][:, 6 * L_L:], func=ACTF.Tanh)
                    i_ap = a_sb[d][:, 0:2 * L_L]
                    f_ap = a_sb[d][:, 2 * L_L:4 * L_L]
                    o_ap = a_sb[d][:, 4 * L_L:6 * L_L]
                    gg_ap = a_sb[d][:, 6 * L_L:8 * L_L]
                    nc.vector.tensor_tensor(out=tg_sb[d][:], in0=i_ap, in1=gg_ap,
                                            op=ALU.mult)
                    nc.vector.tensor_tensor(out=c_sb[d][:], in0=f_ap, in1=c_sb[d][:],
                                            op=ALU.mult)
                    nc.vector.tensor_tensor(out=c_sb[d][:], in0=c_sb[d][:],
                                            in1=tg_sb[d][:], op=ALU.add)
                    nc.scalar.activation(out=tc_sb[d][:], in_=c_sb[d][:],
                                         func=ACTF.Tanh)
                    # h -> scratch (warmup) or houtT strided slice
                    if s < W_L:
                        h_out = h_scr[d][:].rearrange("p (k l) -> p k l", k=2)
                    else:
                        off_o = (s - W_L) if d == 0 else (127 - s)
                        h_out = houtT_v[d][:, :, off_o: off_o + 32 * (L_L - 1) + 1: 32]
                    nc.vector.tensor_tensor(out=h_out, in0=o_ap.rearrange(
                        "p (k l) -> p k l", k=2), in1=tc_sb[d][:].rearrange(
                        "p (k l) -> p k l", k=2), op=ALU.mult)

        # ---- phase 3: feats slabs for viterbi ----
        with tc.tile_pool(name="ps_fe", bufs=4, space="PSUM") as psfe:
            for which, nsl, base_off, dst in ((0, NSF, 0, featsV_f),
                                              (1, NSB, 64, featsV_b)):
                for sl in range(nsl):
                    t0 = base_off + sl
                    pf = psfe.tile([64, K], F32, tag="fe", name="pf")
                    for kc in range(4):
                        d, sub = kc // 2, kc % 2
                        lhs = houtT_v[d][:, sub, t0: t0 + 16 * 63 + 1: 16]
                        nc.tensor.matmul(
                            out=pf[:, :], lhsT=lhs,
                            rhs=w_outT_sb[:, kc * K:(kc + 1) * K],
                            start=(kc == 0), stop=(kc == 3),
                        )
                    nc.scalar.copy(out=dst[0:64, sl * K:(sl + 1) * K], in_=pf[:, :])

        # ---- phase 4: viterbi fv/bv scans ----
        with tc.tile_pool(name="ph4", bufs=1) as p4:
            m_sb = p4.tile([64, K * K], F32)
            mb_sb = p4.tile([64, K * K], F32)
            red_sb = p4.tile([64, K], F32)
            u_sb = p4.tile([64, K], F32)
            fvA = p4.tile([64, K], F32)
            bvA = p4.tile([64, K], F32)
            fvs_sb = p4.tile([64, C_V * K], F32)
            bvs_sb = p4.tile([64, C_V * K], F32)
            nc.gpsimd.memset(fvA[:], 0.0)
            nc.gpsimd.memset(bvA[:], 0.0)

            m3 = m_sb[:].rearrange("l (a b) -> l a b", a=K)
            mb3 = mb_sb[:].rearrange("l (a b) -> l a b", a=K)
            tr3 = trans_sb[0:64, :].rearrange("l (a b) -> l a b", a=K)
            trT3 = transT_sb[0:64, :].rearrange("l (a b) -> l a b", a=K)

            fv_prev = fvA[:, :]
            for s in range(S_V):
                if s == W_V:
                    nc.vector.tensor_tensor(out=fv_prev, in0=fv_prev,
                                            in1=vmask_sb[0:64, 0:K], op=ALU.mult)
                    nc.vector.tensor_tensor(out=fv_prev, in0=fv_prev,
                                            in1=vval_sb[0:64, 0:K], op=ALU.add)
                nc.vector.tensor_tensor(out=m3, in0=_bcast_mid(fv_prev, K),
                                        in1=tr3, op=ALU.add)
                nc.vector.tensor_reduce(out=red_sb[:, :], in_=m3,
                                        axis=mybir.AxisListType.X, op=ALU.max)
                nxt = fvs_sb[:, (s - W_V) * K:(s - W_V + 1) * K] if s >= W_V \
                    else fvA[:, :]
                nc.vector.tensor_tensor(out=nxt, in0=red_sb[:, :],
                                        in1=featsV_f[0:64, s * K:(s + 1) * K],
                                        op=ALU.add)
                fv_prev = nxt

            bv_prev = bvA[:, :]
            for s in range(S_V):
                ftb = featsV_b[0:64, (80 - s) * K:(81 - s) * K]
                nc.vector.tensor_tensor(out=u_sb[:, :], in0=bv_prev, in1=ftb,
                                        op=ALU.add)
                nc.vector.tensor_tensor(out=mb3, in0=_bcast_mid(u_sb[:, :], K),
                                        in1=trT3, op=ALU.add)
                nxt = bvs_sb[:, (79 - s) * K:(80 - s) * K] if s >= W_V \
                    else bvA[:, :]
                nc.vector.tensor_reduce(out=nxt, in_=mb3,
                                        axis=mybir.AxisListType.X, op=ALU.max)
                if s == W_V:
                    nc.vector.tensor_tensor(out=nxt, in0=nxt,
                                            in1=vmask_sb[0:64, K:2 * K], op=ALU.mult)
                    nc.vector.tensor_tensor(out=nxt, in0=nxt,
                                            in1=vval_sb[0:64, K:2 * K], op=ALU.add)
                bv_prev = nxt

            # ---- phase 5: outputs ----
            nc.sync.dma_start(
                out=feats_o.rearrange("(l s) j -> l s j", l=64),
                in_=featsV_b[0:64, 0:C_V * K].rearrange("l (s j) -> l s j", s=C_V))
            nc.sync.dma_start(
                out=fv_o.rearrange("(l s) j -> l s j", l=64),
                in_=fvs_sb[:, :].rearrange("l (s j) -> l s j", s=C_V))
            nc.sync.dma_start(
                out=bv_o.rearrange("(l s) j -> l s j", l=64),
                in_=bvs_sb[:, :].rearrange("l (s j) -> l s j", s=C_V))

    nc.compile()
    return nc


def _get_runner(nc):
    """Build the sharded jitted executor once per process (the per-call
    rebuild inside run_bass_kernel_spmd re-traces/compiles every time)."""
    if "runner" in _PROGRAM_CACHE:
        return _PROGRAM_CACHE["runner"]
    import jax
    from jax.experimental.shard_map import shard_map
    from jax.sharding import Mesh, PartitionSpec
    from concourse import bass2jax

    bass2jax.install_neuronx_cc_hook()
    partition_name = (nc.partition_id_tensor.name
                      if nc.partition_id_tensor else None)
    in_names, out_names, out_avals, zero_shapes = [], [], [], []
    for alloc in nc.m.functions[0].allocations:
        if not isinstance(alloc, mybir.MemoryLocationSet):
            continue
        name = alloc.memorylocations[0].name
        if alloc.kind == "ExternalInput":
            if name != partition_name:
                in_names.append(name)
        elif alloc.kind == "ExternalOutput":
            out_names.append(name)
            shape = tuple(alloc.tensor_shape)
            dtype = mybir.dt.np(alloc.dtype)
            out_avals.append(jax.core.ShapedArray(shape, dtype))
            zero_shapes.append((shape, dtype))
    n_params = len(in_names)
    n_outs = len(out_names)
    all_in_names = in_names + out_names
    if partition_name is not None:
        all_in_names = all_in_names + [partition_name]

    def _body(*targs):
        operands = list(targs)
        if partition_name is not None:
            operands.append(bass2jax.partition_id_tensor())
        res = bass2jax._bass_exec_p.bind(
            *operands,
            out_avals=tuple(out_avals),
            in_names=tuple(all_in_names),
            out_names=tuple(out_names),
            lowering_input_output_aliases=(),
            sim_require_finite=True,
            sim_require_nnan=True,
            nc=nc,
        )
        return tuple(res)

    devices = jax.devices()[:NCORES]
    mesh = Mesh(np.asarray(devices), ("core",))
    in_specs = (PartitionSpec("core"),) * (n_params + n_outs)
    out_specs = (PartitionSpec("core"),) * n_outs
    donate = tuple(range(n_params, n_params + n_outs))
    sharded = jax.jit(
        shard_map(_body, mesh=mesh, in_specs=in_specs, out_specs=out_specs,
                  check_rep=False),
        donate_argnums=donate, keep_unused=True,
    )
    runner = (sharded, in_names, out_names, zero_shapes)
    _PROGRAM_CACHE["runner"] = runner
    return runner


def _run_cached(nc, in_maps):
    sharded, in_names, out_names, zero_shapes = _get_runner(nc)
    concat_in = [
        np.concatenate([np.asarray(in_maps[c][name]) for c in range(NCORES)], axis=0)
        for name in in_names
    ]
    concat_zeros = [np.zeros((NCORES * s[0], *s[1:]), dt) for (s, dt) in zero_shapes]
    out_arrs = sharded(*concat_in, *concat_zeros)
    return {name: np.asarray(out_arrs[i]) for i, name in enumerate(out_names)}


def _prep_core(c, sentence, emb, w_ih_f, w_hh_f, b_f, w_ih_b, w_hh_b, b_b,
               w_out, b_out, transitions, h0, c0):
    base = c * TCORE
    tglob = np.arange(NROWS) + base - 160
    tok = sentence[np.clip(tglob, 0, T - 1)]
    uniq, inv = np.unique(tok, return_inverse=True)
    emb_sub = np.zeros((NROWS, D), np.float32)
    emb_sub[:len(uniq)] = emb[uniq]
    ids = inv.astype(np.int32)

    def packT(wm, pad_k):
        # wm [rows, cols] -> [128, nk*1024] with [p, kc*1024 + slot*128 + r]
        nk = (wm.shape[1] + 127) // 128
        outa = np.zeros((128, nk * 1024), np.float32)
        for kc in range(nk):
            w = min(128, wm.shape[1] - kc * 128)
            for slot in range(8):
                rows = wm[SLOT_ORIG[slot] * 128:(SLOT_ORIG[slot] + 1) * 128,
                          kc * 128: kc * 128 + w]
                outa[:w, kc * 1024 + slot * 128: kc * 1024 + (slot + 1) * 128] = rows.T
        return outa

    w_ihT = np.concatenate([packT(w_ih_f, 3), packT(w_ih_b, 3)], axis=1)
    whhT = np.concatenate([packT(w_hh_f, 2), packT(w_hh_b, 2)], axis=1)
    bias = np.zeros((128, 16), np.float32)
    for d, b in ((0, b_f), (1, b_b)):
        for slot in range(8):
            bias[:, d * 8 + slot] = b[SLOT_ORIG[slot] * 128:(SLOT_ORIG[slot] + 1) * 128]
    w_outT = np.zeros((128, 4 * K), np.float32)
    for kc in range(4):
        w_outT[:, kc * K:(kc + 1) * K] = w_out[:, kc * 128:(kc + 1) * 128].T
    transp = (transitions + b_out[:, None]).astype(np.float32)
    trans_rep = np.broadcast_to(transp.reshape(-1), (128, K * K)).copy()
    transT_rep = np.broadcast_to(transp.T.reshape(-1), (128, K * K)).copy()

    lmask = np.ones((128, 4 * L_L), np.float32)
    lhval = np.zeros((128, 4 * L_L), np.float32)
    lcval = np.zeros((128, 4 * L_L), np.float32)
    if c == 0:
        for kc in range(2):
            lmask[:, 0 * 72 + kc * L_L + 2] = 0.0
            lhval[:, 0 * 72 + kc * L_L + 2] = h0[0][kc * 128:(kc + 1) * 128]
            lcval[:, 0 * 72 + kc * L_L + 2] = c0[0][kc * 128:(kc + 1) * 128]
    if c == NCORES - 1:
        for kc in range(2):
            lmask[:, 1 * 72 + kc * L_L + 33] = 0.0
            lhval[:, 1 * 72 + kc * L_L + 33] = h0[1][kc * 128:(kc + 1) * 128]
            lcval[:, 1 * 72 + kc * L_L + 33] = c0[1][kc * 128:(kc + 1) * 128]

    vmask = np.ones((128, 2 * K), np.float32)
    vval = np.zeros((128, 2 * K), np.float32)
    if c == 0:
        vmask[0, 0:K] = 0.0
        vval[0, 0:K] = NEG
        vval[0, START] = 0.0
    if c == NCORES - 1:
        vmask[63, K:2 * K] = 0.0
        vval[63, K:2 * K] = transitions[STOP]

    return dict(emb_sub=emb_sub, ids=ids, w_ihT=w_ihT, whhT=whhT, bias=bias,
                w_outT=w_outT, trans_rep=trans_rep, transT_rep=transT_rep,
                lmask=lmask, lhval=lhval, lcval=lcval, vmask=vmask, vval=vval)


def kernel(sentence, emb, w_ih_f, w_hh_f, b_f, w_ih_b, w_hh_b, b_b,
           w_out, b_out, transitions, h0, c0):
    sentence = np.asarray(sentence)
    emb = np.asarray(emb, np.float32)
    args = (sentence, emb, np.asarray(w_ih_f), np.asarray(w_hh_f),
            np.asarray(b_f), np.asarray(w_ih_b), np.asarray(w_hh_b),
            np.asarray(b_b), np.asarray(w_out), np.asarray(b_out),
            np.asarray(transitions), np.asarray(h0), np.asarray(c0))

    if "nc" not in _PROGRAM_CACHE:
        _PROGRAM_CACHE["nc"] = _build_program()
    nc = _PROGRAM_CACHE["nc"]

    in_maps = [_prep_core(c, *args) for c in range(NCORES)]
    outs = _run_cached(nc, in_maps)

    feats = outs["feats_o"].reshape(T, K)
    fv = outs["fv_o"].reshape(T, K)
    bv = outs["bv_o"].reshape(T, K)

    b_out_np = np.asarray(b_out, np.float32)
    trans_np = np.asarray(transitions, np.float32)
    feats = feats + b_out_np[None, :]

    tot = fv + bv
    path = tot.argmax(axis=1).astype(np.int32)

    # path score accumulated in the reference's order/precision
    terms = np.empty(2 * T, np.float32)
    prev = np.concatenate([[START], path[:-1]])
    terms[0::2] = trans_np[path, prev]
    terms[1::2] = feats[np.arange(T), path]
    acc = np.cumsum(terms, dtype=np.float32)[-1]
    score = np.float32(acc + trans_np[STOP, path[-1]])
    return score, path
